# revision 13
# baseline (speedup 1.0000x reference)
"""Multi-head attention kernel for Trainium2 (8 NeuronCores, data-parallel over batch).

v4: baseline schedule with the three input projections (Q, K, V) switched to
error-compensated 3-term fp8e4 DoubleRow matmuls:

    out = xh@wh + xl@wh + xh@wl      (hi/lo splits host-prepped, e4m3)

DoubleRow processes a 256-deep contraction at 0.5 cycles/output-row, so each
projection drops from 8 to 6 effective bf16-instruction-equivalents per
256-deep pair: 65536 -> 49152 PE cycles per projection.

Weights are pre-scaled by 32 on the host so their residuals stay out of
e4m3's subnormal range (W ~ U(-1/32,1/32) residuals would underflow).
Consequences of the 32x scale:
  qt/kt hold 32*(Q|K) in bf16  -> exp scale = SCALE/1024
  vext holds 32*V in bf16, ones column memset to 32.0 -> the rowsum divide
  cancels the scale exactly.

Scores / AV / out-proj stay bf16: fp8 on those paths costs 2-3.5% max-rel
error (measured) vs the 2e-2 gate, and 3-term compensation there is not
cheaper than bf16 (64-deep score contraction wastes DoubleRow k-tile slots).

Device pipeline (per core, batch element b) — as the baseline:
  V phase first (DMA-paced), then 8 groups interleaving next-chunk Q/K
  projection, AV of head pair (lag 2), scores+exp of current head pair;
  O evict via DVE reciprocal+scale, OT via xbar DMA transpose, token-major
  out-projection streamed straight to DRAM.
"""

from contextlib import ExitStack

import numpy as np

import concourse.bass as bass
import concourse.mybir as mybir
import concourse.tile as tile
from concourse import bacc
from concourse.bass_utils import run_bass_kernel_spmd

F32 = mybir.dt.float32
BF = mybir.dt.bfloat16
FP8 = mybir.dt.float8e4
DR = mybir.MatmulPerfMode.DoubleRow
ALU = mybir.AluOpType
ACTF = mybir.ActivationFunctionType

B, T, D, H = 8, 1024, 1024, 16
HD = D // H
SCALE = HD**-0.5
WS = 32.0  # host-side weight pre-scale (power of 2)
ESCALE = SCALE / (WS * WS)
P = 128
PT = D // P  # 8 chunks
HE = HD + 1  # 65
DE = H * HE  # 1040


def _build(esc_bufs=24, av_bufs=2, sc_bufs=2, pj_bufs=2, qt_bufs=3, ysb_bufs=3):
    nc = bacc.Bacc(None, target_bir_lowering=False)
    xqh_d = nc.dram_tensor("xqh", [4, P, 2 * T], FP8, kind="ExternalInput")
    xql_d = nc.dram_tensor("xql", [4, P, 2 * T], FP8, kind="ExternalInput")
    xkh_d = nc.dram_tensor("xkh", [4, P, 2 * T], FP8, kind="ExternalInput")
    xkl_d = nc.dram_tensor("xkl", [4, P, 2 * T], FP8, kind="ExternalInput")
    xvh_d = nc.dram_tensor("xvh", [4, P, 2 * T], FP8, kind="ExternalInput")
    xvl_d = nc.dram_tensor("xvl", [4, P, 2 * T], FP8, kind="ExternalInput")
    wqh_d = nc.dram_tensor("wqh", [4, P, 2 * D], FP8, kind="ExternalInput")
    wql_d = nc.dram_tensor("wql", [4, P, 2 * D], FP8, kind="ExternalInput")
    wkh_d = nc.dram_tensor("wkh", [4, P, 2 * D], FP8, kind="ExternalInput")
    wkl_d = nc.dram_tensor("wkl", [4, P, 2 * D], FP8, kind="ExternalInput")
    wvh_d = nc.dram_tensor("wvh", [4, P, 2 * D], FP8, kind="ExternalInput")
    wvl_d = nc.dram_tensor("wvl", [4, P, 2 * D], FP8, kind="ExternalInput")
    wot_d = nc.dram_tensor("wot", [D, D], BF, kind="ExternalInput")
    bq_d = nc.dram_tensor("bq", [D], F32, kind="ExternalInput")  # 32*bq
    bk_d = nc.dram_tensor("bk", [D], F32, kind="ExternalInput")  # 32*bk
    bvh_d = nc.dram_tensor("bvh", [D], BF, kind="ExternalInput")  # 32*bv
    boh_d = nc.dram_tensor("boh", [D], BF, kind="ExternalInput")
    y_d = nc.dram_tensor("y", [T, D], F32, kind="ExternalOutput")

    with tile.TileContext(nc) as tc, ExitStack() as top:
        consts = top.enter_context(tc.tile_pool(name="consts", bufs=1, side="left"))
        bqT = consts.tile([P, PT], F32, tag="bqT")
        nc.gpsimd.dma_start(out=bqT, in_=bq_d[:].rearrange("(k p) -> p k", p=P))
        bkT = consts.tile([P, PT], F32, tag="bkT")
        nc.gpsimd.dma_start(out=bkT, in_=bk_d[:].rearrange("(k p) -> p k", p=P))
        bvb = consts.tile([P, D], BF, tag="bvb")
        nc.gpsimd.dma_start(
            out=bvb, in_=bass.AP(tensor=bvh_d, offset=0, ap=[[0, P], [1, D]])
        )
        bob = consts.tile([P, D], BF, tag="bob")
        nc.gpsimd.dma_start(
            out=bob, in_=bass.AP(tensor=boh_d, offset=0, ap=[[0, P], [1, D]])
        )

        ident = consts.tile([P, P], BF, tag="ident")
        from concourse.masks import make_identity

        make_identity(nc, ident)

        # persistent left pools
        vext_pool = top.enter_context(tc.tile_pool(name="vext", bufs=PT, side="left"))
        vext = [
            vext_pool.tile([P, DE], BF, tag="vext", name=f"vext{i}") for i in range(PT)
        ]
        for k in range(PT):
            # ones column at 32.0: cancels the 32x V scale in the rowsum divide
            nc.gpsimd.memset(
                vext[k].rearrange("p (h x) -> p h x", x=HE)[:, :, HD:HE], WS
            )
        otb_pool = top.enter_context(tc.tile_pool(name="otb", bufs=1, side="left"))
        otb = otb_pool.tile([P, PT * T], BF, tag="otb", name="otb")
        otb3 = otb.rearrange("p (k t) -> p k t", t=T)

        # streaming pools (right side)
        stream = top.enter_context(tc.tile_pool(name="stream", bufs=16, side="right"))
        qkp = top.enter_context(tc.tile_pool(name="qkp", bufs=32, side="right"))
        qt_pool = top.enter_context(tc.tile_pool(name="qt", bufs=qt_bufs, side="right"))
        kt_pool = top.enter_context(tc.tile_pool(name="kt", bufs=qt_bufs, side="right"))
        esc_pool = top.enter_context(
            tc.tile_pool(name="esc", bufs=esc_bufs, side="right")
        )
        obq_pool = top.enter_context(tc.tile_pool(name="obq", bufs=2, side="right"))
        smalls = top.enter_context(tc.tile_pool(name="smalls", bufs=1, side="right"))
        ps = top.enter_context(tc.tile_pool(name="ps", bufs=1, space="PSUM"))

        def r2(t):
            return t.rearrange("p (two t) -> p two t", two=2)

        # ---- input DMA (SP queue, in consumption order) ----
        # V stream: hi tensors first (enable the 2/3 hi terms early), then lo.
        xvh, xvl, wvh, wvl = [], [], [], []
        for c2 in range(4):
            t1 = stream.tile([P, 2 * T], FP8, tag="xw", bufs=8, name=f"xvh{c2}")
            nc.sync.dma_start(out=t1, in_=xvh_d[c2, :, :])
            t2 = stream.tile([P, 2 * D], FP8, tag="xw", bufs=8, name=f"wvh{c2}")
            nc.sync.dma_start(out=t2, in_=wvh_d[c2, :, :])
            xvh.append(r2(t1))
            wvh.append(r2(t2))
        for c2 in range(4):
            t1 = stream.tile([P, 2 * T], FP8, tag="xwl", bufs=8, name=f"xvl{c2}")
            nc.sync.dma_start(out=t1, in_=xvl_d[c2, :, :])
            t2 = stream.tile([P, 2 * D], FP8, tag="xwl", bufs=8, name=f"wvl{c2}")
            nc.sync.dma_start(out=t2, in_=wvl_d[c2, :, :])
            xvl.append(r2(t1))
            wvl.append(r2(t2))

        qk_tiles = {}
        for nm, dram in (
            ("xqh", xqh_d),
            ("wqh", wqh_d),
            ("xql", xql_d),
            ("wql", wql_d),
            ("xkh", xkh_d),
            ("wkh", wkh_d),
            ("xkl", xkl_d),
            ("wkl", wkl_d),
        ):
            lst = []
            for c2 in range(4):
                tt = qkp.tile(
                    [P, dram.shape[2]], FP8, tag="qk", name=f"{nm}{c2}"
                )
                nc.sync.dma_start(out=tt, in_=dram[c2, :, :])
                lst.append(r2(tt))
            qk_tiles[nm] = lst

        # ---- V phase: V_ext[s, i_ext] token-major, 3-term fp8 ----
        # 4 concurrent PSUM groups; stationary = xv s-block, moving = wv i-half
        for base in range(0, 16, 4):
            pvs = []
            for g in range(4):
                if g < 2:
                    pv = ps.tile([P, 512], F32, tag="pj", bufs=pj_bufs, name=f"pv{base}_{g}")
                else:
                    pvt = ps.tile([P, T], F32, tag="sc", bufs=sc_bufs, name=f"pv{base}_{g}")
                    pv = pvt[:, 0:512]
                pvs.append(pv)
            for c2 in range(4):
                for ti, (xs, ws) in enumerate(
                    ((xvh, wvh), (xvl, wvh), (xvh, wvl))
                ):
                    for g in range(4):
                        ci, k = (base + g) // 8, (base + g) % 8
                        nc.tensor.matmul(
                            pvs[g],
                            xs[c2][:, :, 128 * k : 128 * (k + 1)],
                            ws[c2][:, :, 512 * ci : 512 * (ci + 1)],
                            start=(c2 == 0 and ti == 0),
                            stop=(c2 == 3 and ti == 2),
                            perf_mode=DR,
                        )
            for g in range(4):
                ci, k = (base + g) // 8, (base + g) % 8
                nc.vector.tensor_tensor(
                    out=vext[k].rearrange("p (h x) -> p h x", x=HE)[
                        :, 8 * ci : 8 * (ci + 1), 0:HD
                    ],
                    in0=pvs[g].rearrange("p (h x) -> p h x", x=HD),
                    in1=bvb[:, 512 * ci : 512 * (ci + 1)].rearrange(
                        "p (h x) -> p h x", x=HD
                    ),
                    op=ALU.add,
                )

        # wo tiles reuse the V stream slots (DMA waits on V-phase readers)
        wo = []
        for j in range(PT):
            wt = stream.tile([P, D], BF, tag="xw" if j < 8 else "xwl", bufs=8, name=f"wo{j}")
            nc.sync.dma_start(out=wt, in_=wot_d[j * P : (j + 1) * P, :])
            wo.append(wt)

        qt = {}
        kt = {}

        def proj_items(dst, k, wh, wl, xh, xl, bias):
            """4 closures; each emits 6 of the 12 DoubleRow instrs of one
            512-col projection (c2-pairs 01 / 23)."""
            items = []
            state = {}

            def mk(c, half):
                def run():
                    if half == 0:
                        state[c] = ps.tile([P, 512], F32, tag="pj", bufs=pj_bufs, name=f"pj{k}_{c}")
                    pt_ = state[c]
                    for c2 in range(2 * half, 2 * half + 2):
                        for ti, (ws_, xs_) in enumerate(
                            ((wh, xh), (wh, xl), (wl, xh))
                        ):
                            nc.tensor.matmul(
                                pt_[:, :],
                                ws_[c2][:, :, 128 * k : 128 * (k + 1)],
                                xs_[c2][:, :, 512 * c : 512 * (c + 1)],
                                start=(c2 == 0 and ti == 0),
                                stop=(c2 == 3 and ti == 2),
                                perf_mode=DR,
                            )
                    if half == 1:
                        nc.vector.tensor_scalar(
                            out=dst[:, 512 * c : 512 * (c + 1)],
                            in0=pt_[:, :],
                            scalar1=bias[:, k : k + 1],
                            scalar2=None,
                            op0=ALU.add,
                        )

                return run

            for c in range(2):
                items.append(mk(c, 0))
                items.append(mk(c, 1))
            return items

        def make_qk_items(k):
            qt[k] = qt_pool.tile([P, T], BF, tag="qt", name=f"qt{k}")
            kt[k] = kt_pool.tile([P, T], BF, tag="kt", name=f"kt{k}")
            return proj_items(
                qt[k], k, qk_tiles["wqh"], qk_tiles["wql"], qk_tiles["xqh"], qk_tiles["xql"], bqT
            ) + proj_items(
                kt[k], k, qk_tiles["wkh"], qk_tiles["wkl"], qk_tiles["xkh"], qk_tiles["xkl"], bkT
            )

        esc = {}  # head -> list of 8 esc tiles

        def make_sc_items(h):
            hi, ro = h // 2, 64 * (h % 2)
            esc[h] = []

            def mk(s):
                def run():
                    psc = ps.tile([P, T], F32, tag="sc", bufs=sc_bufs, name=f"sc{h}_{s}")
                    for c in range(2):
                        nc.tensor.matmul(
                            psc[:, 512 * c : 512 * (c + 1)],
                            kt[hi][ro : ro + 64, 128 * s : 128 * (s + 1)],
                            qt[hi][ro : ro + 64, 512 * c : 512 * (c + 1)],
                            start=True,
                            stop=True,
                        )
                    e = esc_pool.tile([P, T], BF, tag="esc", name=f"esc{h}_{s}")
                    nc.scalar.activation(out=e, in_=psc[:, :], func=ACTF.Exp, scale=ESCALE)
                    esc[h].append(e)

                return run

            return [mk(s) for s in range(PT)]

        obq = {}  # quad -> tile [P, PT, 256]

        def make_av_items(h):
            q = h // 4
            if q not in obq:
                t_ = obq_pool.tile([P, PT * 256], BF, tag="ob", bufs=2, name=f"ob{q}")
                obq[q] = t_.rearrange("p (t i) -> p t i", i=256)
            ob = obq[q]
            col = 64 * (h % 4)

            def mk(tm):
                def run():
                    pav = ps.tile([P, HE], F32, tag="av", bufs=av_bufs, name=f"av{h}_{tm}")
                    for s in range(PT):
                        nc.tensor.matmul(
                            pav[:, :],
                            esc[h][s][:, 128 * tm : 128 * (tm + 1)],
                            vext[s][:, HE * h : HE * (h + 1)],
                            start=(s == 0),
                            stop=(s == PT - 1),
                            skip_group_check=True,
                        )
                    rcp = smalls.tile([P, 1], F32, tag="rcp", bufs=6, name=f"rcp{h}_{tm}")
                    nc.vector.reciprocal(rcp, pav[:, HD : HD + 1])
                    nc.vector.tensor_scalar(
                        out=ob[:, tm, col : col + HD],
                        in0=pav[:, 0:HD],
                        scalar1=rcp,
                        scalar2=None,
                        op0=ALU.mult,
                    )
                    if h % 2 == 1:
                        p_ = h // 2
                        if h == H - 1:
                            # final pair: PE transposes (ACT still streaming)
                            tps = tail_ps[:, 64 * tm : 64 * (tm + 1)].bitcast(BF)
                            nc.tensor.transpose(
                                tps,
                                ob[:, tm, 128 * (p_ % 2) : 128 * (p_ % 2) + 128],
                                ident,
                            )
                            nc.vector.tensor_copy(
                                otb3[:, p_, 128 * tm : 128 * (tm + 1)], tps
                            )
                        else:
                            nc.sync.dma_start_transpose(
                                out=otb3[:, p_, 128 * tm : 128 * (tm + 1)],
                                in_=ob[:, tm, 128 * (p_ % 2) : 128 * (p_ % 2) + 128],
                            )

                return run

            return [mk(tm) for tm in range(PT)]

        # ---- pre-loop: QT(0)/KT(0) ----
        for it in make_qk_items(0):
            it()

        # ---- attention groups ----
        for k in range(PT):
            sc_items = make_sc_items(2 * k) + make_sc_items(2 * k + 1)
            qk_items = make_qk_items(k + 1) if k < PT - 1 else []
            av_items = (
                make_av_items(2 * k - 2) + make_av_items(2 * k - 1) if k >= 1 else []
            )
            for i in range(16):
                sc_items[i]()
                if av_items:
                    av_items[i]()
                if qk_items and i % 2 == 0:
                    qk_items[i // 2]()

        # tail AVs (heads 14, 15)
        tail_ps = ps.tile([P, T], F32, tag="sc", bufs=sc_bufs, name="tail_ps")
        for it in make_av_items(14) + make_av_items(15):
            it()

        # ---- output projection (token-major, direct DMA out) ----
        for c in range(2):
            for m in range(PT):
                if m % 2 == 0:
                    psy = ps.tile([P, 512], F32, tag="pj", bufs=pj_bufs, name=f"py{c}_{m}")
                else:
                    pyt = ps.tile([P, T], F32, tag="sc", bufs=sc_bufs, name=f"py{c}_{m}")
                    psy = pyt[:, 0:512]
                for k in range(PT):
                    nc.tensor.matmul(
                        psy,
                        otb3[:, k, 128 * m : 128 * (m + 1)],
                        wo[k][:, 512 * c : 512 * (c + 1)],
                        start=(k == 0),
                        stop=(k == PT - 1),
                    )
                ysb = smalls.tile([P, 512], F32, tag="ysb", bufs=ysb_bufs, name=f"ysb{c}_{m}")
                nc.vector.tensor_tensor(
                    out=ysb,
                    in0=psy,
                    in1=bob[:, 512 * c : 512 * (c + 1)],
                    op=ALU.add,
                )
                nc.scalar.dma_start(
                    out=y_d[128 * m : 128 * (m + 1), 512 * c : 512 * (c + 1)],
                    in_=ysb,
                )

    nc.compile()
    return nc


_NC_CACHE = None


def _get_nc():
    global _NC_CACHE
    if _NC_CACHE is None:
        _NC_CACHE = _build()
    return _NC_CACHE


def _pairs(a):
    """[1024, n] -> [4, 128, 2n]: d-chunk pairs, k-halves along free dim."""
    n = a.shape[1]
    return np.ascontiguousarray(
        a.reshape(4, 2, 128, n).transpose(0, 2, 1, 3).reshape(4, 128, 2 * n)
    )


def kernel(**inputs) -> np.ndarray:
    import ml_dtypes

    bf16 = ml_dtypes.bfloat16
    e4m3 = ml_dtypes.float8_e4m3

    def split_pairs(a):  # [d, n] f32 -> hi, lo packed pair tiles (e4m3)
        hi = a.astype(e4m3)
        lo = (a - hi.astype(np.float32)).astype(e4m3)
        return _pairs(hi), _pairs(lo)

    query = np.asarray(inputs["query"], dtype=np.float32)
    key = np.asarray(inputs["key"], dtype=np.float32)
    value = np.asarray(inputs["value"], dtype=np.float32)

    wqh, wql = split_pairs(np.asarray(inputs["Wq"], np.float32).T * WS)
    wkh, wkl = split_pairs(np.asarray(inputs["Wk"], np.float32).T * WS)
    wvh, wvl = split_pairs(np.asarray(inputs["Wv"], np.float32).T * WS)
    wot = np.ascontiguousarray(np.asarray(inputs["Wo"], np.float32).T).astype(bf16)

    bq = np.ascontiguousarray(np.asarray(inputs["bq"], np.float32) * WS)
    bk = np.ascontiguousarray(np.asarray(inputs["bk"], np.float32) * WS)
    bvh = (np.asarray(inputs["bv"], np.float32) * WS).astype(bf16)
    boh = np.asarray(inputs["bo"], np.float32).astype(bf16)

    nc = _get_nc()
    in_maps = []
    for b in range(B):
        xqh, xql = split_pairs(np.ascontiguousarray(query[b].T))
        xkh, xkl = split_pairs(np.ascontiguousarray(key[b].T))
        xvh, xvl = split_pairs(np.ascontiguousarray(value[b].T))
        in_maps.append(
            {
                "xqh": xqh, "xql": xql,
                "xkh": xkh, "xkl": xkl,
                "xvh": xvh, "xvl": xvl,
                "wqh": wqh, "wql": wql,
                "wkh": wkh, "wkl": wkl,
                "wvh": wvh, "wvl": wvl,
                "wot": wot,
                "bq": bq, "bk": bk, "bvh": bvh, "boh": boh,
            }
        )
    res = run_bass_kernel_spmd(nc, in_maps, core_ids=list(range(B)))
    return np.stack([res.results[b]["y"] for b in range(B)], axis=0)
